# revision 25
# baseline (speedup 1.0000x reference)
"""Position-attention kernel for Trainium2 (8 NeuronCores, Bass/Tile).

Module: q,k = 1x1 convs to C/8 channels, v = 1x1 conv, attn = softmax(q^T k),
y = v @ attn^T, out = gamma*y + x.  Shapes: B=4, C=512, H=W=64 (N=4096, Cq=64).

Sharding: data-parallel over batch x query-halves -> 8 cores. Core i handles
batch i//2, query positions [h*2048, (h+1)*2048) with h = i%2. Each core
computes full K/V projections for its batch (duplicated across the pair) and
its half of Q, then S^T = k^T q in [key m, query n] layout (so no transposes
are needed anywhere), exp, and y = v @ attn^T via vT-stationary matmuls with
a ones-row matmul accumulating the softmax denominator. Normalization and the
gamma*y + x residual are fused into the epilogue.

Host-side folds: weights pre-transposed + cast to bf16; gamma folded into v_w;
gamma*v_b folded into the residual (softmax rows sum to 1); per-core key
permutation puts the core's own query half first so one SPMD program works for
both halves.
"""

import numpy as np
import ml_dtypes

import concourse.bass as bass
import concourse.mybir as mybir
import concourse.tile as tile
from concourse import bacc, bass_isa
from concourse.bass_utils import run_bass_kernel_spmd

BF16 = ml_dtypes.bfloat16

B, C, H, W = 4, 512, 64, 64
N = H * W            # 4096 keys per batch
NQ = N // 2          # 2048 queries per core
CQ = C // 8          # 64 q/k channels
P = 128
CT = C // P          # 4 channel tiles
MT = N // P          # 32 key tiles
NCH = 512            # matmul moving-dim chunk
QCH = NQ // NCH      # 4 query chunks per core
KCH = N // NCH       # 8 key chunks
NCORES = 8

F32 = mybir.dt.float32
BF = mybir.dt.bfloat16
F8 = mybir.dt.float8e4
F8E = mybir.dt.float8e5
AF = mybir.ActivationFunctionType
LN16 = 2.772588722239781  # exp shift (ln 16): E in fp8e5m2, max logit ~10.9 -> e^8.1 ~ 3300 < 57344

_CACHE = {}


def _build_program():
    # Bacc (not raw Bass): its finalize() runs generate_event_semaphores,
    # which splits multi-semaphore waits — walrus codegen allows only one
    # sync wait per instruction.
    nc = bacc.Bacc()

    xb = nc.declare_dram_parameter("xb", [C, N], BF, isOutput=False)
    xr = nc.declare_dram_parameter("xr", [C, NQ], F32, isOutput=False)
    qw = nc.declare_dram_parameter("qw", [C, CQ], BF, isOutput=False)
    kw = nc.declare_dram_parameter("kw", [C, CQ], BF, isOutput=False)
    vw = nc.declare_dram_parameter("vw", [C, C], BF, isOutput=False)
    qb = nc.declare_dram_parameter("qb", [CQ, 1], F32, isOutput=False)
    kb = nc.declare_dram_parameter("kb", [CQ, 1], F32, isOutput=False)
    out = nc.declare_dram_parameter("out", [C, NQ], F32, isOutput=True)

    with tile.TileContext(nc) as tc:
        with tc.tile_pool(name="consts", bufs=1) as consts:
            x_sb = consts.tile([P, CT * N], BF)        # x[b] as 4 c-tiles side by side
            qw_sb = consts.tile([P, CT * CQ], BF)
            kw_sb = consts.tile([P, CT * CQ], BF)
            vw_sb = consts.tile([P, CT * C], BF)
            qb_sb = consts.tile([CQ, 1], F32)
            kb_sb = consts.tile([CQ, 1], F32)
            xr_sb = consts.tile([P, CT * NQ], F32)     # residual (+ gamma*v_b) slice
            q_sb = consts.tile([CQ, NQ], BF)
            k_sb = consts.tile([CQ, N], BF)
            vt_sb = consts.tile([P, MT * C], F8)       # vT: 32 m-tiles of [128, 512]

            # Consolidated input DMAs: one instruction per tensor (rearranged
            # APs cover all 4 c-tiles) — each dma_start costs ~0.6us of
            # sequencer descriptor-gen, so fewer + split across the two HWDGE
            # queues (sync, scalar). x is split so its first 512 columns land
            # before the rest streams in.
            xb_r = xb[:, :].rearrange("(t p) m -> p t m", p=P)
            xsb_r = x_sb.rearrange("p (t m) -> p t m", t=CT)
            kw_r = kw[:, :].rearrange("(t p) o -> p t o", p=P)
            qw_r = qw[:, :].rearrange("(t p) o -> p t o", p=P)
            vw_r = vw[:, :].rearrange("(t p) o -> p t o", p=P)
            nc.sync.dma_start(out=kw_sb.rearrange("p (t o) -> p t o", t=CT), in_=kw_r)
            nc.scalar.dma_start(out=qw_sb.rearrange("p (t o) -> p t o", t=CT), in_=qw_r)
            nc.scalar.dma_start(out=kb_sb, in_=kb[:, :])
            nc.scalar.dma_start(out=qb_sb, in_=qb[:, :])
            nc.sync.dma_start(out=xsb_r[:, :, :NCH], in_=xb_r[:, :, :NCH])
            nc.scalar.dma_start(out=xsb_r[:, 2:, NCH:NQ], in_=xb_r[:, 2:, NCH:NQ])
            nc.sync.dma_start(out=xsb_r[:, :2, NCH:NQ], in_=xb_r[:, :2, NCH:NQ])
            nc.scalar.dma_start(out=vw_sb.rearrange("p (t o) -> p t o", t=CT), in_=vw_r)
            nc.sync.dma_start(out=xsb_r[:, :2, NQ:], in_=xb_r[:, :2, NQ:])
            nc.scalar.dma_start(out=xsb_r[:, 2:, NQ:], in_=xb_r[:, 2:, NQ:])
            # Touch the bias tiles on ACT before the matmul stream: the
            # Activation-with-bias struct only has one sync-wait slot, so the
            # real bias copies must not need a separate DMA wait.
            bias_touch = consts.tile([CQ, 2], F32)
            nc.scalar.activation(bias_touch[:, 0:1], kb_sb, AF.Copy)
            nc.scalar.activation(bias_touch[:, 1:2], qb_sb, AF.Copy)
            ln16_sb = consts.tile([P, 1], F32)
            nc.vector.memset(ln16_sb, -LN16)

            # ---- projections ----
            # Emitted in x-column-arrival order: work needing only the first
            # 512 columns first, then 512:2048, then the rest.
            with tc.tile_pool(name="proj_ps", bufs=2, space="PSUM") as proj_ps:
                def k_proj(ch):
                    kp = proj_ps.tile([CQ, NCH], F32, tag="kq", name="kp")
                    for ct in range(CT):
                        nc.tensor.matmul(
                            kp,
                            lhsT=kw_sb[:, ct * CQ:(ct + 1) * CQ],
                            rhs=x_sb[:, ct * N + ch * NCH: ct * N + (ch + 1) * NCH],
                            start=(ct == 0), stop=(ct == CT - 1))
                    nc.scalar.activation(k_sb[:, ch * NCH:(ch + 1) * NCH], kp,
                                         AF.Identity, bias=kb_sb)

                def q_proj(ch):
                    qp = proj_ps.tile([CQ, NCH], F32, tag="kq", name="qp")
                    for ct in range(CT):
                        nc.tensor.matmul(
                            qp,
                            lhsT=qw_sb[:, ct * CQ:(ct + 1) * CQ],
                            rhs=x_sb[:, ct * N + ch * NCH: ct * N + (ch + 1) * NCH],
                            start=(ct == 0), stop=(ct == CT - 1))
                    nc.scalar.activation(q_sb[:, ch * NCH:(ch + 1) * NCH], qp,
                                         AF.Identity, bias=qb_sb)

                def v_proj(mt):
                    vp = proj_ps.tile([P, C], F32, tag="v", name="vp")
                    for ct in range(CT):
                        nc.tensor.matmul(
                            vp,
                            lhsT=x_sb[:, ct * N + mt * P: ct * N + (mt + 1) * P],
                            rhs=vw_sb[:, ct * C:(ct + 1) * C],
                            start=(ct == 0), stop=(ct == CT - 1))
                    nc.scalar.activation(vt_sb[:, mt * C:(mt + 1) * C], vp, AF.Copy)

                k_proj(0); q_proj(0)
                for mt in range(4):
                    v_proj(mt)
                for ch in range(1, 4):
                    k_proj(ch); q_proj(ch)
                for mt in range(4, 16):
                    v_proj(mt)
                for ch in range(4, KCH):
                    k_proj(ch)
                for mt in range(16, MT):
                    v_proj(mt)

            nc.scalar.dma_start(out=xr_sb.rearrange("p (t m) -> p t m", t=CT),
                                in_=xr[:, :].rearrange("(t p) m -> p t m", p=P))

            # ---- attention main loop ----
            with (
                tc.tile_pool(name="u_ps", bufs=1, space="PSUM") as u_ps,
                tc.tile_pool(name="st_ps", bufs=4, space="PSUM") as st_ps,
                tc.tile_pool(name="e_pool", bufs=8) as e_pool,
                tc.tile_pool(name="cs_pool", bufs=2) as cs_pool,
                tc.tile_pool(name="fin", bufs=2) as fin,
                tc.tile_pool(name="outp", bufs=4) as outp,
            ):
                for ch in range(QCH):
                    u = u_ps.tile([P, CT * NCH], F32, tag="u", name="u")
                    cs_acc = cs_pool.tile([P, 2 * NCH], F32, tag="cs_acc", name="cs_acc")
                    qs = q_sb[:, ch * NCH:(ch + 1) * NCH]

                    sts = {}

                    def emit_st(mt, _qs=qs):
                        st = st_ps.tile([P, NCH], F32, tag="st", name="st")
                        nc.tensor.matmul(st, lhsT=k_sb[:, mt * P:(mt + 1) * P],
                                         rhs=_qs, start=True, stop=True)
                        sts[mt] = st

                    emit_st(0)
                    emit_st(1)
                    emit_st(2)
                    vt_r = vt_sb.rearrange("p (m c) -> p m c", m=MT)
                    for t in range(MT // 2):
                        e2 = e_pool.tile([P, 2, NCH], F8E, tag="e", name="e2")
                        for j in range(2):
                            mt = 2 * t + j
                            nc.scalar.activation(e2[:, j, :], sts.pop(mt), AF.Exp,
                                                 bias=ln16_sb)
                            if mt + 3 < MT:
                                emit_st(mt + 3)
                        e2f = e2.rearrange("p j n -> p (j n)")
                        if t == 0:
                            nc.vector.tensor_copy(cs_acc, e2f)
                        else:
                            nc.vector.tensor_add(cs_acc, cs_acc, e2f)
                        for c in range(CT):
                            nc.tensor.matmul(
                                u[:, c * NCH:(c + 1) * NCH],
                                lhsT=vt_r[:, 2 * t:2 * t + 2, c * P:(c + 1) * P],
                                rhs=e2,
                                start=(t == 0), stop=(t == MT // 2 - 1),
                                perf_mode=mybir.MatmulPerfMode.DoubleRow)

                    last = ch == QCH - 1
                    if not last:
                        # Drain U out of PSUM on ACT (one wide op) so the PE
                        # can start the next chunk without waiting.
                        uc = outp.tile([P, CT * NCH], F32, tag="uc", name="uc")
                        nc.scalar.activation(uc, u, AF.Copy)
                        u_src = uc
                    else:
                        # Final chunk: nothing follows — DVE reads U straight
                        # from PSUM to shorten the tail.
                        u_src = u
                    # fold the pair halves, reduce over partitions, reciprocal
                    csf = fin.tile([P, NCH], F32, tag="csf", name="csf")
                    nc.vector.tensor_add(csf, cs_acc[:, :NCH], cs_acc[:, NCH:])
                    csr = fin.tile([P, NCH], F32, tag="csr", name="csr")
                    nc.gpsimd.partition_all_reduce(
                        csr, csf, channels=P, reduce_op=bass_isa.ReduceOp.add)
                    rec = fin.tile([P, NCH], F32, tag="rec", name="rec")
                    nc.vector.reciprocal_approx_fast(out=rec, in_=csr)
                    rec_b = bass.AP(tensor=rec.tensor, offset=rec.offset,
                                    ap=[rec.ap[0], [0, CT], rec.ap[1]])
                    o = outp.tile([P, CT * NCH], F32, tag="o", name="o")
                    o3 = o.rearrange("p (c n) -> p c n", c=CT)
                    u3 = u_src.rearrange("p (c n) -> p c n", c=CT)
                    xr3 = xr_sb.rearrange("p (c m) -> p c m", c=CT)[
                        :, :, ch * NCH:(ch + 1) * NCH]
                    nc.vector.tensor_mul(o3, u3, rec_b)
                    nc.vector.tensor_add(o3, o3, xr3)
                    out_r = out[:, :].rearrange("(c p) n -> p c n", p=P)
                    nc.sync.dma_start(out=out_r[:, :, ch * NCH:(ch + 1) * NCH],
                                      in_=o3)
    nc.finalize()
    return nc


def _get_program():
    if "nc" not in _CACHE:
        _CACHE["nc"] = _build_program()
    return _CACHE["nc"]


def make_in_maps(x, q_w, q_b, k_w, k_b, v_w, v_b, gamma):
    x = np.asarray(x, dtype=np.float32)
    gamma_f = float(np.asarray(gamma).reshape(-1)[0])
    qwT = np.ascontiguousarray(np.asarray(q_w, np.float32).T).astype(BF16)
    kwT = np.ascontiguousarray(np.asarray(k_w, np.float32).T).astype(BF16)
    vwT = np.ascontiguousarray(gamma_f * np.asarray(v_w, np.float32).T).astype(BF16)
    qb_c = np.asarray(q_b, np.float32).reshape(CQ, 1)
    kb_c = np.asarray(k_b, np.float32).reshape(CQ, 1)
    gvb = (gamma_f * np.asarray(v_b, np.float32)).reshape(C, 1)

    xf = x.reshape(B, C, N)
    in_maps = []
    for core in range(NCORES):
        b, h = core // 2, core % 2
        mine = xf[b, :, h * NQ:(h + 1) * NQ]
        other = xf[b, :, (1 - h) * NQ:(2 - h) * NQ]
        x_perm = np.concatenate([mine, other], axis=1)
        in_maps.append({
            "xb": x_perm.astype(BF16),
            "xr": np.ascontiguousarray(mine) + gvb,
            "qw": qwT, "kw": kwT, "vw": vwT,
            "qb": qb_c, "kb": kb_c,
        })
    return in_maps


def run(in_maps, **kwargs):
    nc = _get_program()
    return run_bass_kernel_spmd(nc, in_maps, list(range(NCORES)), **kwargs)


def gather(results):
    out = np.empty((B, C, N), dtype=np.float32)
    for core in range(NCORES):
        b, h = core // 2, core % 2
        out[b, :, h * NQ:(h + 1) * NQ] = results[core]["out"]
    return out.reshape(B, C, H, W)


def kernel(x, q_w, q_b, k_w, k_b, v_w, v_b, gamma, **_):
    in_maps = make_in_maps(x, q_w, q_b, k_w, k_b, v_w, v_b, gamma)
    res = run(in_maps)
    return gather(res.results)
